# revision 11
# baseline (speedup 1.0000x reference)
"""Multi-head causal attention (B=2, S=2048, D=4096, H=32, hd=128) on 8 trn2 cores.

Sharding: DP over batch (2) x TP over heads (4 groups of 8 heads).
Core c: batch b = c//4, head-group tp = c%4.
Each core computes a partial output [2048, 4096] (wo row-sharded); host sums
the 4 partials per batch.

All matmuls run as float32r (full PE rate, ~tf32 precision).
Host pre-transposes x / weights / mask so every DMA is natural-layout.
q/k head dims are de-interleaved (evens then odds) on the host so RoPE becomes
full-tile DVE ops on partition halves; the permutation is consistent between
q and k so scores are unchanged. v / wo stay in natural order.
Scores are computed transposed ([tk, tq]) so the PV matmul needs no
on-chip transpose of the probabilities; softmax is unnormalized exp with the
denominator from a ones-vector matmul, divided into the attention output.
"""

import sys
sys.path.insert(0, '/opt/trn_rl_repo')
sys.path.insert(0, '/opt/trn_rl_repo/concourse')

import numpy as np

S = 2048
D = 4096
HD = 128
FSH = 1024            # features per core (8 heads)
NHL = 8               # heads per core
KT = D // 128         # 32 k-tiles for projections
TSTRIPS = S // 512    # 4 tq strips
NKT = S // 128        # 16 tk tiles
NEG_THRESH = -1.0e8

_cache = {}


def _build(classes):
    """Build + compile the per-core Bacc program. classes[j][s] in {0:skip,1:zero,2:add}."""
    import concourse.bacc as bacc
    import concourse.mybir as mybir
    import concourse.tile as tile

    f32 = mybir.dt.float32
    f32r = mybir.dt.float32r
    EXP = mybir.ActivationFunctionType.Exp

    nc = bacc.Bacc("TRN2", target_bir_lowering=False, debug=False)

    xt_d = nc.dram_tensor("xt", [D, S], f32r, kind="ExternalInput").ap()
    wqt_d = nc.dram_tensor("wqt", [D, FSH], f32r, kind="ExternalInput").ap()
    wkt_d = nc.dram_tensor("wkt", [D, FSH], f32r, kind="ExternalInput").ap()
    wvt_d = nc.dram_tensor("wvt", [D, FSH], f32r, kind="ExternalInput").ap()
    wot_d = nc.dram_tensor("wot", [FSH, D], f32r, kind="ExternalInput").ap()
    cos_d = nc.dram_tensor("cosw", [64, S], f32, kind="ExternalInput").ap()
    sin_d = nc.dram_tensor("sinw", [64, S], f32, kind="ExternalInput").ap()
    nsin_d = nc.dram_tensor("nsinw", [64, S], f32, kind="ExternalInput").ap()
    mask_d = nc.dram_tensor("maskt", [S, S], f32, kind="ExternalInput").ap()
    id_d = nc.dram_tensor("id128", [128, 128], f32r, kind="ExternalInput").ap()
    on_d = nc.dram_tensor("ones128", [128, 128], f32r, kind="ExternalInput").ap()
    out_d = nc.dram_tensor("out", [S, D], f32, kind="ExternalOutput").ap()

    with tile.TileContext(nc) as tc, \
         nc.allow_low_precision(reason="float32r is 4-byte near-fp32"):
        with tc.tile_pool(name="pdram", bufs=1, space="DRAM") as pdram, \
             tc.tile_pool(name="pconst", bufs=1) as pconst:
            qt_d = pdram.tile([FSH, S], f32r, name="qt_spill")
            kt_d = pdram.tile([FSH, S], f32r, name="kt_spill")
            vt_d = pdram.tile([FSH, S], f32r, name="vt_spill")
            att_d = pdram.tile([FSH, S], f32r, name="att_spill")
            ones_sb = pconst.tile([128, 128], f32r, name="ones_sb")
            nc.sync.dma_start(out=ones_sb, in_=on_d)
            ones_k = ones_sb[:, 0:1]
            ones_c = ones_sb[0:1, :]
            id_sb = pconst.tile([128, 128], f32r, name="id_sb")
            nc.sync.dma_start(out=id_sb, in_=id_d)

            # ---------------- Phase 1: q/k/v projections (+RoPE on q,k) -------------
            with tc.tile_pool(name="p1x", bufs=KT) as p1x, \
                 tc.tile_pool(name="p1w", bufs=2) as p1w, \
                 tc.tile_pool(name="p1t", bufs=4) as p1t, \
                 tc.tile_pool(name="p1o", bufs=4) as p1o, \
                 tc.tile_pool(name="p1cs", bufs=1) as p1cs, \
                 tc.tile_pool(name="ps1", bufs=4, space="PSUM") as ps1:
                for T2 in range(2):           # t-strips of 1024
                    t0 = T2 * 1024
                    xk = []
                    for k in range(KT):
                        xt_t = p1x.tile([128, 1024], f32r, name="xk")
                        nc.sync.dma_start(out=xt_t, in_=xt_d[k * 128:(k + 1) * 128, t0:t0 + 1024])
                        xk.append(xt_t)
                    cos_sb = p1cs.tile([64, 1024], f32, name="cos_sb")
                    sin_sb = p1cs.tile([64, 1024], f32, name="sin_sb")
                    nsin_sb = p1cs.tile([64, 1024], f32, name="nsin_sb")
                    nc.sync.dma_start(out=cos_sb, in_=cos_d[:, t0:t0 + 1024])
                    nc.sync.dma_start(out=sin_sb, in_=sin_d[:, t0:t0 + 1024])
                    nc.sync.dma_start(out=nsin_sb, in_=nsin_d[:, t0:t0 + 1024])

                    for proj, (w_d, spill) in enumerate(
                            [(wqt_d, qt_d), (wkt_d, kt_d), (wvt_d, vt_d)]):
                        for i in range(NHL):  # 8 f-tiles of 128
                            wt = p1w.tile([128, KT, 128], f32r, name="wt")
                            w_ap = w_d[:, i * 128:(i + 1) * 128].rearrange(
                                "(k p) f -> p k f", p=128)
                            nc.sync.dma_start(out=wt, in_=w_ap)
                            for tsub in range(2):
                                ps = ps1.tile([128, 512], f32, name="ps1")
                                for k in range(KT):
                                    nc.tensor.matmul(
                                        ps, wt[:, k, :],
                                        xk[k][:, tsub * 512:(tsub + 1) * 512],
                                        start=(k == 0), stop=(k == KT - 1))
                                ot = p1o.tile([128, 512], f32r, name="ot")
                                csl = slice(tsub * 512, (tsub + 1) * 512)
                                if proj < 2:  # RoPE for q, k
                                    m1 = p1t.tile([64, 512], f32, name="m1")
                                    m2 = p1t.tile([64, 512], f32, name="m2")
                                    nc.vector.tensor_mul(m1, ps[0:64], cos_sb[:, csl])
                                    nc.vector.tensor_mul(m2, ps[64:128], nsin_sb[:, csl])
                                    nc.vector.tensor_add(ot[0:64], m1, m2)
                                    m3 = p1t.tile([64, 512], f32, name="m1")
                                    m4 = p1t.tile([64, 512], f32, name="m2")
                                    nc.vector.tensor_mul(m3, ps[0:64], sin_sb[:, csl])
                                    nc.vector.tensor_mul(m4, ps[64:128], cos_sb[:, csl])
                                    nc.vector.tensor_add(ot[64:128], m3, m4)
                                else:
                                    nc.vector.tensor_copy(ot, ps)
                                nc.sync.dma_start(
                                    out=spill[i * 128:(i + 1) * 128,
                                              t0 + tsub * 512:t0 + (tsub + 1) * 512],
                                    in_=ot)

            # ---------------- Phase 2: attention per head ----------------------------
            with tc.tile_pool(name="p2h", bufs=2) as p2h, \
                 tc.tile_pool(name="p2v", bufs=NKT + 1) as p2v, \
                 tc.tile_pool(name="p2e", bufs=4) as p2e, \
                 tc.tile_pool(name="p2m", bufs=3) as p2m, \
                 tc.tile_pool(name="p2r", bufs=4) as p2r, \
                 tc.tile_pool(name="p2o", bufs=4) as p2o, \
                 tc.tile_pool(name="ps2s", bufs=2, space="PSUM") as ps2s, \
                 tc.tile_pool(name="ps2a", bufs=2, space="PSUM") as ps2a, \
                 tc.tile_pool(name="ps2t", bufs=2, space="PSUM") as ps2t, \
                 tc.tile_pool(name="ps2d", bufs=2, space="PSUM") as ps2d:
                for h in range(NHL):
                    kt_h = p2h.tile([128, S], f32r, name="kt_h")
                    qt_h = p2h.tile([128, S], f32r, name="qt_h")
                    vt_h = p2h.tile([128, S], f32r, name="vt_h")
                    nc.sync.dma_start(out=kt_h, in_=kt_d[h * 128:(h + 1) * 128, :])
                    nc.sync.dma_start(out=qt_h, in_=qt_d[h * 128:(h + 1) * 128, :])
                    nc.sync.dma_start(out=vt_h, in_=vt_d[h * 128:(h + 1) * 128, :])
                    v_sb = []
                    for j in range(NKT):
                        tps = ps2t.tile([128, 128], f32r, name="tp")
                        nc.tensor.transpose(tps, vt_h[:, j * 128:(j + 1) * 128], id_sb)
                        vj = p2v.tile([128, 128], f32r, name="vj")
                        nc.vector.tensor_copy(vj, tps)
                        v_sb.append(vj)
                    for s in range(TSTRIPS):
                        act = [j for j in range(NKT) if classes[j][s] != 0]
                        A = ps2a.tile([128, 512], f32, name="A")
                        Dn = ps2d.tile([1, 512], f32, name="Dn")
                        qs = qt_h[:, s * 512:(s + 1) * 512]
                        for idx, j in enumerate(act):
                            sps = ps2s.tile([128, 512], f32, name="sps")
                            nc.tensor.matmul(sps, kt_h[:, j * 128:(j + 1) * 128], qs,
                                             start=True, stop=True)
                            E = p2e.tile([128, 512], f32r, name="E")
                            if classes[j][s] == 2:
                                mt = p2m.tile([128, 512], f32, name="mt")
                                nc.sync.dma_start(
                                    out=mt,
                                    in_=mask_d[j * 128:(j + 1) * 128, s * 512:(s + 1) * 512])
                                ms = p2m.tile([128, 512], f32, name="ms")
                                nc.vector.tensor_add(ms, sps, mt)
                                nc.scalar.activation(E, ms, EXP)
                            else:
                                nc.scalar.activation(E, sps, EXP)
                            first, last = (idx == 0), (idx == len(act) - 1)
                            nc.tensor.matmul(A, v_sb[j], E, start=first, stop=last)
                            nc.tensor.matmul(Dn, ones_k, E, start=first, stop=last)
                        rec = p2r.tile([1, 512], f32r, name="rec")
                        nc.vector.reciprocal(rec, Dn[0:1, :])
                        bps = ps2s.tile([128, 512], f32, name="sps")
                        nc.tensor.matmul(bps, ones_c, rec, start=True, stop=True)
                        bsb = p2o.tile([128, 512], f32, name="bsb")
                        nc.vector.tensor_copy(bsb, bps)
                        ao = p2o.tile([128, 512], f32r, name="ao")
                        nc.vector.tensor_mul(ao, A, bsb)
                        nc.sync.dma_start(
                            out=att_d[h * 128:(h + 1) * 128, s * 512:(s + 1) * 512],
                            in_=ao)

            # ---------------- Phase 3: output projection ------------------------------
            with tc.tile_pool(name="p3w", bufs=8) as p3w, \
                 tc.tile_pool(name="p3a", bufs=NHL + 1) as p3a, \
                 tc.tile_pool(name="p3o", bufs=4) as p3o, \
                 tc.tile_pool(name="ps3", bufs=4, space="PSUM") as ps3:
                wts = []
                for c in range(8):        # dout chunks of 512; all resident
                    wt = p3w.tile([128, NHL, 512], f32r, name="w3")
                    w_ap = wot_d[:, c * 512:(c + 1) * 512].rearrange(
                        "(k p) f -> p k f", p=128)
                    nc.sync.dma_start(out=wt, in_=w_ap)
                    wts.append(wt)
                for m in range(NKT):      # t tiles of 128
                    am = []
                    for k in range(NHL):
                        a_t = p3a.tile([128, 128], f32r, name="a3")
                        nc.sync.dma_start(
                            out=a_t,
                            in_=att_d[k * 128:(k + 1) * 128, m * 128:(m + 1) * 128])
                        am.append(a_t)
                    for c in range(8):
                        ps = ps3.tile([128, 512], f32, name="ps3")
                        for k in range(NHL):
                            nc.tensor.matmul(ps, am[k], wts[c][:, k, :],
                                             start=(k == 0), stop=(k == NHL - 1))
                        ot = p3o.tile([128, 512], f32, name="o3")
                        nc.vector.tensor_copy(ot, ps)
                        nc.sync.dma_start(
                            out=out_d[m * 128:(m + 1) * 128, c * 512:(c + 1) * 512],
                            in_=ot)

    nc.compile()
    return nc


def _host_prep(x, wq, wk, wv, wo, freqs_cos, freqs_sin, mask):
    """Build per-core input maps + mask block classes."""
    x = np.asarray(x, np.float32)
    wq = np.asarray(wq, np.float32)
    wk = np.asarray(wk, np.float32)
    wv = np.asarray(wv, np.float32)
    wo = np.asarray(wo, np.float32)
    mask2 = np.asarray(mask, np.float32).reshape(S, S)

    perm = np.concatenate(
        [hl * 128 + np.concatenate([np.arange(0, 128, 2), np.arange(1, 128, 2)])
         for hl in range(NHL)])
    cosw = np.ascontiguousarray(np.asarray(freqs_cos, np.float32).T)
    sinw = np.ascontiguousarray(np.asarray(freqs_sin, np.float32).T)
    nsinw = np.ascontiguousarray(-sinw)
    maskt = np.ascontiguousarray(mask2.T)
    id128 = np.eye(128, dtype=np.float32)

    classes = [[0] * TSTRIPS for _ in range(NKT)]
    for j in range(NKT):
        for s in range(TSTRIPS):
            blk = maskt[j * 128:(j + 1) * 128, s * 512:(s + 1) * 512]
            if (blk <= NEG_THRESH).all():
                classes[j][s] = 0
            elif (blk == 0.0).all():
                classes[j][s] = 1
            else:
                classes[j][s] = 2

    xts = [np.ascontiguousarray(x[b].T) for b in range(2)]
    in_maps = []
    for core in range(8):
        b, tp = core // 4, core % 4
        sl = slice(tp * FSH, (tp + 1) * FSH)
        wq_c = wq[sl][perm] * np.float32(1.0 / np.sqrt(HD))
        wk_c = wk[sl][perm]
        in_maps.append({
            "xt": xts[b],
            "wqt": np.ascontiguousarray(wq_c.T),
            "wkt": np.ascontiguousarray(wk_c.T),
            "wvt": np.ascontiguousarray(wv[sl].T),
            "wot": np.ascontiguousarray(wo[:, sl].T),
            "cosw": cosw, "sinw": sinw, "nsinw": nsinw,
            "maskt": maskt, "id128": id128,
            "ones128": np.ones((128, 128), np.float32),
        })
    return in_maps, classes


def kernel(x, wq, wk, wv, wo, freqs_cos, freqs_sin, mask, start_pos=0,
           _trace=False):
    from concourse import bass_utils
    in_maps, classes = _host_prep(x, wq, wk, wv, wo, freqs_cos, freqs_sin, mask)
    key = str(classes)
    if key not in _cache:
        _cache[key] = _build(classes)
    nc = _cache[key]
    res = bass_utils.run_bass_kernel_spmd(nc, in_maps, core_ids=list(range(8)),
                                          trace=_trace)
    out = np.zeros((2, S, D), np.float32)
    for core in range(8):
        out[core // 4] += res.results[core]["out"]
    kernel.last_result = res
    return out


if __name__ == "__main__":
    # compile-only smoke test
    classes = [[2 if j * 128 <= s * 512 + 511 and j * 128 + 127 > s * 512 else
                (1 if j * 128 + 127 <= s * 512 else 0)
                for s in range(TSTRIPS)] for j in range(NKT)]
    import time
    t0 = time.time()
    nc = _build(classes)
    print(f"build+bacc-compile: {time.time()-t0:.1f}s")
    if len(sys.argv) > 1 and sys.argv[1] == "neff":
        import tempfile
        from concourse import bass_utils
        t0 = time.time()
        with tempfile.TemporaryDirectory() as td:
            bass_utils.compile_bass_kernel(nc, td)
            print(f"walrus: {time.time()-t0:.1f}s COMPILED OK")
